# revision 5
# baseline (speedup 1.0000x reference)
"""AggregationMPNN Trainium2 kernel (data-parallel over the graph/batch dim).

Math (per graph, matching the reference):
  hidden = zeropad(nodes)                                [V, H]
  3x message pass:
    att_p[w,e,m] = hidden[w] @ att_W[e]; msg_p likewise  (biases are zero)
    Because edges[v,w,:] is one-hot (masked), softmax attention collapses to
      numer[v,m] = sum_{w,e} edges[v,w,e] * exp(att_p[w,e,m]) * msg_p[w,e,m]
      denom[v,m] = sum_{w,e} edges[v,w,e] * exp(att_p[w,e,m])
      message    = numer / (denom + 1e-30)
    GRU update, applied only where node degree > 0 (denom > 0).
  readout: sum_v sigmoid([h,nodes]@Wa) * (h@We) * mask

Layout: 8 graphs/core => 512 node slots. Hidden is kept TRANSPOSED in SBUF
(hT[H=256, 512]) feeding projections as lhsT and the GRU as rhs. All matmul
operands are bf16 (edges fp8e4: 0/1 are exact) with fp32 PSUM accumulation.
sigmoid(x) is computed as 0.5*tanh(0.5x)+0.5 so every activation uses the
exp_and_others table set; the 0.5/0.5 affine fixups are folded away (Wnh
pre-halved on host, mask stored as 0.5/0, matching 2x in the readout gate).
The per-pair edge gather is one matmul per edge type over a block-diagonal
[128,128] edge tile; the denominator eps rides in as a rank-1 matmul.

Perf structure:
  - inputs land via 7 large DMAs split across the two HWDGE queues (sync +
    scalar) so issue overhead (~0.6us each) overlaps; pass-0-critical tensors
    go first on each queue.
  - ~3.5us of throwaway matmuls at kernel start keep the PE HAM activity
    window busy during the input DMA so the clock is at 8/8 when real work
    starts (otherwise the first ~25us run at 1.2 GHz).
  - GRU matmuls are ordered hidden-contractions first, message-contractions
    last, so the PE never queues behind the msgN -> transpose -> copy chain
    that produces msgT.
"""

import sys

sys.path.insert(0, "/opt/trn_rl_repo")

import numpy as np

N, V, E, NF, H, M = 64, 64, 8, 64, 256, 128
OUT = H
NCORES = 8
G = N // NCORES          # graphs per core
VG = V * G               # node slots per core (512)
NPAIR = G // 2           # graph pairs per core (4)
EPS = 1e-30
MASK_THRESH = 1e-20      # denom > thresh <=> node has a neighbour (real
                         # denoms are >= exp(min att) >> 1e-20; eps = 1e-30)
HCOL = 256               # node columns per pipeline half (2 graph pairs)

_BUILT = None            # cached compiled bass module
TRACE = False            # test.py sets kernel.TRACE = True for profiling
LAST_RESULTS = None      # BassKernelResults of the last run (for profiling)


def _emit(ctx, tc, d, npasses=3, dbg=False):
    import concourse.bass as bass  # noqa: F401
    from concourse import mybir
    from concourse.masks import make_identity

    nc = tc.nc
    FP = mybir.dt.float32
    BF = mybir.dt.bfloat16
    F8 = mybir.dt.float8e4
    AF = mybir.ActivationFunctionType
    OP = mybir.AluOpType
    AX = mybir.AxisListType

    def mm(out, lhsT, rhs, start, stop):
        nc.tensor.matmul(out, lhsT, rhs, start=start, stop=stop)

    consts = ctx.enter_context(tc.tile_pool(name="consts", bufs=1))
    work = ctx.enter_context(tc.tile_pool(name="work", bufs=3))
    pp_ps = ctx.enter_context(tc.tile_pool(name="pp_ps", bufs=3, space="PSUM"))
    gat_ps = ctx.enter_context(tc.tile_pool(name="gat_ps", bufs=2, space="PSUM"))
    gru_ps = ctx.enter_context(tc.tile_pool(name="gru_ps", bufs=3, space="PSUM"))

    # ---- persistent SBUF state ----
    hT0 = consts.tile([128, 2, HCOL], BF)       # hidden^T, node cols 0:256
    hT1 = consts.tile([128, 2, HCOL], BF)       # hidden^T, node cols 256:512
    hTh = (hT0, hT1)
    nodesT = consts.tile([64, VG], BF)          # nodes^T
    wc = consts.tile([128, 2, 2 * E * M], BF)   # [att | msg] proj weights
    edge = consts.tile([128, NPAIR, E, 128], F8)  # block-diag edges^T per pair
    gw = consts.tile([128, 3584], BF)           # GRU + readout weights, packed
    identB = consts.tile([128, 128], BF)
    epsrow = consts.tile([128, 128], BF)        # row 0 = EPS, rest 0
    ones4 = consts.tile([128, 2, 2, M], BF)     # all-ones rhs for the eps mm
    AB = consts.tile([128, NPAIR, E * 2 * M], BF)   # per e: [A(128) | B(128)]
    msgT0 = consts.tile([128, HCOL], BF)
    msgT1 = consts.tile([128, HCOL], BF)
    msgTh = (msgT0, msgT1)
    maskh = consts.tile([128, 2, VG], BF)       # 0.5*mask, bcast over parts
    red = consts.tile([128, 2, G], FP)

    # views into the packed weight tile (layout matches d["gru0"]/d["late"])
    wrzv = {0: gw[:, 0:512], 2: gw[:, 512:1024], 1: gw[:, 1536:2048]}
    wniv = gw[:, 1024:1280]
    wnhv = {0: gw[:, 1280:1536], 1: gw[:, 2048:2304]}
    wgav = gw[:, 2304:3072].rearrange("p (k c) -> p k c", c=OUT)
    wgev = gw[:, 3072:3584].rearrange("p (k c) -> p k c", c=OUT)

    # ---- input DMAs: few large transfers, split across both HWDGE queues,
    # ordered by when the consuming phase needs them ----
    nc.sync.dma_start(out=nodesT[:], in_=d["nodesT"][:])
    nc.scalar.dma_start(out=wc[0:64, 0, :], in_=d["wc0"][:])
    nc.sync.dma_start(out=edge[:], in_=d["edges_p"][:])
    nc.scalar.dma_start(out=gw[:, 0:1536], in_=d["gru0"][:])
    nc.sync.dma_start(out=wc[64:128, 0, :], in_=d["wc0b"][:])
    nc.scalar.dma_start(out=wc[:, 1, :], in_=d["wc1"][:])
    nc.sync.dma_start(out=gw[:, 1536:3584], in_=d["late"][:])

    def keep_warm(n, wide=False):
        # throwaway matmuls that slot into an upcoming PE-idle window (FIFO
        # order): they hold the HAM activity window open so the clock stays
        # at 8/8 across the dependency stall, and cost nothing while idle
        kw = gru_ps.tile([128, 512], FP, tag="g")
        rhs = ones4[:] if wide else ones4[:, 0, :, :]
        for i in range(n):
            mm(kw[:, 0:rhs.free_size()], epsrow[:], rhs, i == 0, i == n - 1)

    # gpsimd-side init (no DMA dependency) + PE warm-up during the input DMA:
    # ~8 N=512 matmuls ~= 3.4us at the cold 1.2 GHz clock, enough to trip the
    # HAM busy window so real work starts at 2.4 GHz.
    nc.gpsimd.memset(ones4[:], 1.0)
    nc.gpsimd.memset(epsrow[:], 0.0)
    nc.gpsimd.memset(epsrow[0:1, :], EPS)
    keep_warm(8, wide=True)
    make_identity(nc, identB[:])
    # init hidden^T = [nodes^T ; 0] (on GpSimd: DVE stays free for pass 0)
    for i in range(2):
        nc.gpsimd.memset(hTh[i][:], 0.0)
        nc.gpsimd.tensor_copy(out=hTh[i][0:64, 0, :],
                              in_=nodesT[:, i * HCOL:(i + 1) * HCOL])

    def emit_proj(cs, pass0=False):
        # projections + A/B construction, one PSUM bank per (half, cc)
        for c in cs:
            abv = AB[:, c, :].rearrange("p (e x) -> p e x", x=2 * M)
            for half in range(2):        # 0: att (exp->B) | 1: msg (*B->A)
                for cc in range(2):
                    q = half * 2 + cc
                    pp = pp_ps.tile([128, 512], FP, tag="pp")
                    if pass0:
                        mm(pp[:], nodesT[:, c * 128:(c + 1) * 128],
                           wc[0:64, 0, q * 512:(q + 1) * 512], True, True)
                    else:
                        for k in range(2):
                            lh = hTh[c // 2][:, k,
                                             (c % 2) * 128:(c % 2 + 1) * 128]
                            mm(pp[:], lh, wc[:, k, q * 512:(q + 1) * 512],
                               k == 0, k == 1)
                    ppv = pp[:].rearrange("p (e m) -> p e m", m=M)
                    esl = slice(cc * 4, (cc + 1) * 4)
                    if half == 0:
                        nc.scalar.activation(out=abv[:, esl, M:2 * M],
                                             in_=ppv, func=AF.Exp)
                    else:
                        nc.vector.tensor_mul(out=abv[:, esl, 0:M], in0=ppv,
                                             in1=abv[:, esl, M:2 * M])

    def emit_gather(hf, first):
        # one matmul per (pair, edge type) + a rank-1 eps matmul
        gat = gat_ps.tile([128, 2, 2, M], FP, tag="gat")
        # one bank-wide eps matmul starts the group (lazy-zeroes the whole
        # 2KB region); both pairs' edge matmuls then accumulate inside it
        mm(gat[:], epsrow[:], ones4[:], True, False)
        for ci in range(2):
            c = 2 * hf + ci
            for e in range(E):
                mm(gat[:, ci, :, :], edge[:, c, e, :],
                   AB[:, c, e * 2 * M:(e + 1) * 2 * M], False,
                   ci == 1 and e == E - 1)
        rec = work.tile([128, 2, M], FP, tag="rec")
        nc.vector.reciprocal_approx_fast(out=rec[:], in_=gat[:, :, 1, :])
        msgN = work.tile([128, 2, M], BF, tag="msgN")
        nc.vector.tensor_mul(out=msgN[:], in0=gat[:, :, 0, :], in1=rec[:])
        den_sb = None
        if first:
            den_sb = work.tile([128, 2, M], BF, tag="den")
            nc.vector.tensor_scalar(den_sb[:], gat[:, :, 1, :], MASK_THRESH,
                                    0.5, OP.is_gt, OP.mult)
        return msgN, den_sb

    def emit_msgT(hf, msgN, den_sb):
        # mt/dt live in the gather pool: the gat tile's consumers (rec/msgN/
        # den) are exactly these transposes' producers, so same-tag rotation
        # adds no false waits -- and it keeps the gru pool's 3 banks free for
        # ps_r/ghn/ps_z (gin reuses ps_r's bank after rt has drained it)
        sl = slice(hf * HCOL, (hf + 1) * HCOL)
        mt = gat_ps.tile([128, HCOL], BF, tag="gat")
        for ci in range(2):
            nc.tensor.transpose(mt[:, ci * 128:(ci + 1) * 128],
                                msgN[:, ci, :], identB[:])
        nc.vector.tensor_copy(out=msgTh[hf][:], in_=mt[:])
        if den_sb is not None:
            dt = gat_ps.tile([128, HCOL], BF, tag="gat")
            for ci in range(2):
                nc.tensor.transpose(dt[:, ci * 128:(ci + 1) * 128],
                                    den_sb[:, ci, :], identB[:])
            nc.vector.tensor_copy(out=maskh[:, 0, sl], in_=dt[:])
            nc.gpsimd.tensor_copy(out=maskh[:, 1, sl], in_=maskh[:, 0, sl])

    def emit_gru(hf, first, final=False):
        # final=True: the caller consumes u2 via the incremental readout and
        # nothing reads hT afterwards, so the in-place hT update is skipped
        # (and the readout-base matmuls are emitted between the GRU matmuls
        # and the tail chain to keep the PE fed)
        sl = slice(hf * HCOL, (hf + 1) * HCOL)
        hks = (0,) if first else (0, 1)
        # allocation order = bank-rotation order (3 bufs, 4 tiles): gin (the
        # 4th) reuses ps_r's bank, which rt has fully drained by the time the
        # gin matmuls issue -- the other pairings would stall the PE longer
        ps_r = gru_ps.tile([128, 2, HCOL], FP, tag="g")
        ghn = gru_ps.tile([128, 2, HCOL], FP, tag="g")
        ps_z = gru_ps.tile([128, 2, HCOL], FP, tag="g")
        gin = gru_ps.tile([128, 2, HCOL], FP, tag="g")
        # hidden-state contractions first: they depend only on hT, so the PE
        # never waits here for msgT (which trails the gather by the
        # msgN -> transpose -> copy chain). msg contractions close each
        # accumulation group at the end; r's group completes first so the
        # rt -> t1 -> t2 -> tanh tail starts as early as possible.
        # ONE bank-wide start per tile: a second start=True would clear the
        # whole bank's has_written bits while the other jj's group is open.
        for jj in range(2):
            for i, k in enumerate(hks):
                mm(ps_r[:, jj, :], wrzv[k][:, jj * 128:(jj + 1) * 128],
                   hTh[hf][:, k, :], jj == 0 and i == 0, False)
        for jj in range(2):
            for i, k in enumerate(hks):
                mm(ghn[:, jj, :], wnhv[k][:, jj * 128:(jj + 1) * 128],
                   hTh[hf][:, k, :], jj == 0 and i == 0,
                   jj == 1 and i == len(hks) - 1)
        for jj in range(2):
            mm(ps_r[:, jj, :], wrzv[2][:, jj * 128:(jj + 1) * 128],
               msgTh[hf][:], False, jj == 1)
        for jj in range(2):
            for i, k in enumerate(hks):
                mm(ps_z[:, jj, :],
                   wrzv[k][:, 256 + jj * 128:256 + (jj + 1) * 128],
                   hTh[hf][:, k, :], jj == 0 and i == 0, False)
        for jj in range(2):
            mm(ps_z[:, jj, :], wrzv[2][:, 256 + jj * 128:256 + (jj + 1) * 128],
               msgTh[hf][:], False, jj == 1)
        for jj in range(2):
            mm(gin[:, jj, :], wniv[:, jj * 128:(jj + 1) * 128],
               msgTh[hf][:], jj == 0, jj == 1)
        base = None
        if final:
            base = emit_readout_base(hf)
            keep_warm(8)
        # r = sigmoid(x) = 0.5*(tanh(0.5x)+1); Wnh is pre-halved so
        # r*gh_n = (tanh(0.5x)+1) * ghn'
        rt = work.tile([128, 2, HCOL], BF, tag="rt")
        nc.scalar.activation(out=rt[:], in_=ps_r[:], func=AF.Tanh, scale=0.5)
        zt = work.tile([128, 2, HCOL], BF, tag="zt")
        nc.scalar.activation(out=zt[:], in_=ps_z[:], func=AF.Tanh, scale=-0.5)
        # mz = mask*(1-z) = (tanh(-0.5x)+1) * maskh
        # (two ops: Pool has no scalar_tensor_tensor opcode on hardware, and
        # the single-scalar ADD,BYPASS form hits an 8.9us ucode path that
        # also starves concurrent DVE SBUF reads -- use MULTIPLY,ADD)
        zc1 = work.tile([128, 2, HCOL], BF, tag="zc1")
        nc.gpsimd.tensor_scalar(zc1[:], zt[:], 1.0, 1.0, OP.mult, OP.add)
        mz = work.tile([128, 2, HCOL], BF, tag="mz")
        nc.gpsimd.tensor_mul(out=mz[:], in0=zc1[:], in1=maskh[:, :, sl])
        t1 = work.tile([128, 2, HCOL], BF, tag="t1")
        t2 = work.tile([128, 2, HCOL], BF, tag="t2")
        nT = work.tile([128, 2, HCOL], BF, tag="nT")
        dd = work.tile([128, 2, HCOL], BF, tag="dd")
        u2 = work.tile([128, 2, HCOL], BF, tag="u2")
        if not final:
            nc.vector.scalar_tensor_tensor(out=t1[:], in0=rt[:], scalar=1.0,
                                           in1=ghn[:], op0=OP.add, op1=OP.mult)
            nc.vector.tensor_add(out=t2[:], in0=gin[:], in1=t1[:])
            nc.scalar.activation(out=nT[:], in_=t2[:], func=AF.Tanh)
            # h' = h + mz*(n - h)
            nc.vector.tensor_tensor(out=dd[:], in0=nT[:], in1=hTh[hf][:],
                                    op=OP.subtract)
            nc.vector.tensor_mul(out=u2[:], in0=mz[:], in1=dd[:])
            nc.vector.tensor_add(out=hTh[hf][:], in0=hTh[hf][:], in1=u2[:])
            return
        # final half: run the tail per H-chunk so the readout-delta matmuls
        # of chunk 0 keep the PE (and its clock) busy during chunk 1's chain
        gps, eps2 = base
        for k in range(2):
            ck = slice(k, k + 1)
            nc.vector.scalar_tensor_tensor(out=t1[:, ck, :], in0=rt[:, ck, :],
                                           scalar=1.0, in1=ghn[:, ck, :],
                                           op0=OP.add, op1=OP.mult)
            nc.vector.tensor_add(out=t2[:, ck, :], in0=gin[:, ck, :],
                                 in1=t1[:, ck, :])
            nc.scalar.activation(out=nT[:, ck, :], in_=t2[:, ck, :],
                                 func=AF.Tanh)
        for k in range(2):
            ck = slice(k, k + 1)
            nc.vector.tensor_tensor(out=dd[:, ck, :], in0=nT[:, ck, :],
                                    in1=hTh[hf][:, ck, :], op=OP.subtract)
            nc.vector.tensor_mul(out=u2[:, ck, :], in0=mz[:, ck, :],
                                 in1=dd[:, ck, :])
            for jj in range(2):
                mm(gps[:, jj, :], wgav[:, k, jj * 128:(jj + 1) * 128],
                   u2[:, k, :], False, k == 1 and jj == 1)
            for jj in range(2):
                mm(eps2[:, jj, :], wgev[:, k, jj * 128:(jj + 1) * 128],
                   u2[:, k, :], False, k == 1 and jj == 1)
        emit_readout_tail(hf, gps, eps2, split=True)

    def emit_readout_base(hf, stop=False):
        # gate/emb projections of the PRE-update hidden state (and nodes);
        # linear in h, so the h' = h + u2 correction can accumulate later
        # ONE accumulation group per PSUM bank (start lazily zeroes the whole
        # 2KB zero region; both jj column blocks accumulate inside it)
        sl = slice(hf * HCOL, (hf + 1) * HCOL)
        gps = pp_ps.tile([128, 2, HCOL], FP, tag="pp")
        for n, jj in enumerate(range(2)):
            mm(gps[:, jj, :], wgav[:, 0, jj * 128:(jj + 1) * 128],
               hTh[hf][:, 0, :], n == 0, False)
            mm(gps[:, jj, :], wgav[:, 1, jj * 128:(jj + 1) * 128],
               hTh[hf][:, 1, :], False, False)
            mm(gps[:, jj, :], wgav[0:64, 2, jj * 128:(jj + 1) * 128],
               nodesT[:, sl], False, stop and n == 1)
        eps2 = pp_ps.tile([128, 2, HCOL], FP, tag="pp")
        for n, jj in enumerate(range(2)):
            mm(eps2[:, jj, :], wgev[:, 0, jj * 128:(jj + 1) * 128],
               hTh[hf][:, 0, :], n == 0, False)
            mm(eps2[:, jj, :], wgev[:, 1, jj * 128:(jj + 1) * 128],
               hTh[hf][:, 1, :], False, stop and n == 1)
        return gps, eps2

    def emit_readout_tail(hf, gps, eps2, split=False):
        sl = slice(hf * HCOL, (hf + 1) * HCOL)
        # 2*gate = tanh(0.5x)+1 pairs with the 0.5-scaled mask; split=True
        # pipelines the chain per jj chunk (kernel-final critical path)
        gt = work.tile([128, 2, HCOL], BF, tag="rt")
        tt = work.tile([128, 2, HCOL], BF, tag="t1")
        t2r = work.tile([128, 2, HCOL], BF, tag="t2")
        jjs = (slice(0, 1), slice(1, 2)) if split else (slice(0, 2),)
        for cj in jjs:
            nc.scalar.activation(out=gt[:, cj, :], in_=gps[:, cj, :],
                                 func=AF.Tanh, scale=0.5)
            nc.vector.scalar_tensor_tensor(out=tt[:, cj, :], in0=gt[:, cj, :],
                                           scalar=1.0, in1=eps2[:, cj, :],
                                           op0=OP.add, op1=OP.mult)
            nc.vector.tensor_mul(out=t2r[:, cj, :], in0=tt[:, cj, :],
                                 in1=maskh[:, cj, sl])
            nc.vector.tensor_reduce(
                out=red[:, cj, hf * 4:(hf + 1) * 4],
                in_=t2r[:, cj, :].rearrange("p j (g v) -> p j g v", v=V),
                axis=AX.X, op=OP.add)
        # ship this half's sums immediately; the host does the tiny
        # [d,j,g] -> [g, j*128+d] transpose (saves the on-device PE
        # transpose + copy and overlaps the DMA with the other half)
        nc.sync.dma_start(out=d["red"][:, :, hf * 4:(hf + 1) * 4],
                          in_=red[:, :, hf * 4:(hf + 1) * 4])

    def emit_readout(hf):
        gps, eps2 = emit_readout_base(hf, stop=True)
        emit_readout_tail(hf, gps, eps2)


    for p in range(npasses):
        first = p == 0
        last = p == npasses - 1
        if first:
            emit_proj((0, 1, 2, 3), pass0=True)
            msgN0, den0 = emit_gather(0, first)
        else:
            # gather of pairs 0,1 is ready (their AB was finished last pass):
            # emit it before the projections so the PE does not queue behind
            # the previous half's GRU tail waiting for the new hT
            msgN0, den0 = emit_gather(0, first)
            keep_warm(8)
            emit_proj((2, 3))
        emit_msgT(0, msgN0, den0)
        emit_gru(0, first)
        msgN1, den1 = emit_gather(1, first)
        if not last:
            # next pass's first two projection pairs: hT half A is final,
            # and the PE would otherwise idle behind this half's GRU chain
            emit_proj((0, 1))
        else:
            # half 0's readout is ready now; emitting it before half 1's
            # GRU keeps its DVE ops out of the half-1 critical tail chain
            emit_readout(0)
        emit_msgT(1, msgN1, den1)
        emit_gru(1, first, final=last)

        if dbg:
            nc.sync.dma_start(out=d[f"dbg_hT{p}"][:, :, 0:HCOL],
                              in_=hTh[0][:])
            nc.sync.dma_start(out=d[f"dbg_hT{p}"][:, :, HCOL:VG],
                              in_=hTh[1][:])
            if p == 0:
                nc.sync.dma_start(out=d["dbg_AB"][:], in_=AB[:])
                nc.sync.dma_start(out=d["dbg_msgT"][:, 0:HCOL],
                                  in_=msgTh[0][:])
                nc.sync.dma_start(out=d["dbg_msgT"][:, HCOL:VG],
                                  in_=msgTh[1][:])
                nc.sync.dma_start(out=d["dbg_maskb"][:], in_=maskh[:, 0, :])



def build(npasses=3, dbg=False):
    """Build + compile the bass module (cached)."""
    global _BUILT
    if _BUILT is not None and not dbg and npasses == 3:
        return _BUILT
    import concourse.bacc as bacc
    import concourse.tile as tile
    from concourse import mybir

    FP = mybir.dt.float32
    BF = mybir.dt.bfloat16
    F8 = mybir.dt.float8e4
    nc = bacc.Bacc("TRN2", target_bir_lowering=False)
    d = {
        "nodesT": nc.dram_tensor("nodesT", [NF, VG], BF, kind="ExternalInput"),
        "edges_p": nc.dram_tensor("edges_p", [128, NPAIR, E, 128], F8,
                                  kind="ExternalInput"),
        "wc0": nc.dram_tensor("wc0", [64, 2048], BF, kind="ExternalInput"),
        "wc0b": nc.dram_tensor("wc0b", [64, 2048], BF, kind="ExternalInput"),
        "wc1": nc.dram_tensor("wc1", [128, 2048], BF, kind="ExternalInput"),
        "gru0": nc.dram_tensor("gru0", [128, 1536], BF, kind="ExternalInput"),
        "late": nc.dram_tensor("late", [128, 2048], BF, kind="ExternalInput"),
        "red": nc.dram_tensor("red", [128, 2, G], FP, kind="ExternalOutput"),
    }
    if dbg:
        for name, shape in [
            ("dbg_AB", [128, NPAIR, E * 2 * M]),
            ("dbg_msgT", [128, VG]),
            ("dbg_maskb", [128, VG]),
        ] + [(f"dbg_hT{p}", [128, 2, VG]) for p in range(npasses)]:
            d[name] = nc.dram_tensor(name, shape, BF, kind="ExternalOutput")
    from contextlib import ExitStack

    with tile.TileContext(nc) as tc:
        with ExitStack() as ctx:
            _emit(ctx, tc, d, npasses=npasses, dbg=dbg)
    nc.compile()
    if not dbg and npasses == 3:
        _BUILT = nc
    return nc


def make_in_maps(nodes, edges, msg_W, msg_b, att_W, att_b, gru_W_ih, gru_W_hh,
                 gru_b_ih, gru_b_hh, gather_att_W, gather_att_b, gather_emb_W,
                 gather_emb_b):
    """Host-side layout prep (transposes/concats/bf16 cast) + sharding."""
    import ml_dtypes

    bf = ml_dtypes.bfloat16
    f8 = ml_dtypes.float8_e4m3
    for b in (msg_b, att_b, gru_b_ih, gru_b_hh, gather_att_b, gather_emb_b):
        if np.abs(np.asarray(b)).max() > 0:
            raise NotImplementedError("nonzero biases not folded on device")
    wc = np.concatenate([
        np.ascontiguousarray(att_W.transpose(1, 0, 2)).reshape(H, E * M),
        np.ascontiguousarray(msg_W.transpose(1, 0, 2)).reshape(H, E * M),
    ], axis=1).astype(np.float32)
    wrz = np.concatenate([gru_W_hh[:2 * H].T, gru_W_ih[:2 * H].T],
                         axis=0).astype(np.float32)          # [384, 512]
    wnh = 0.5 * np.asarray(gru_W_hh[2 * H:].T, np.float32)   # [256, 256]
    wni = np.asarray(gru_W_ih[2 * H:].T, np.float32)         # [128, 256]
    wga = np.zeros((128, 3, OUT), np.float32)                # [128, 3, 256]
    wga[:, 0] = gather_att_W[0:128]
    wga[:, 1] = gather_att_W[128:256]
    wga[0:64, 2] = gather_att_W[256:320]
    wge = np.asarray(gather_emb_W, np.float32).reshape(2, 128, OUT)
    wge = np.ascontiguousarray(wge.transpose(1, 0, 2))       # [128, 2, 256]
    gru0 = np.concatenate([wrz[0:128], wrz[256:384], wni, wnh[0:128]],
                          axis=1)                            # [128, 1536]
    late = np.concatenate([wrz[128:256], wnh[128:256],
                           wga.reshape(128, 768), wge.reshape(128, 512)],
                          axis=1)                            # [128, 2048]
    shared = {
        "wc0": np.ascontiguousarray(wc[0:64]).astype(bf),
        "wc0b": np.ascontiguousarray(wc[64:128]).astype(bf),
        "wc1": np.ascontiguousarray(wc[128:256]).astype(bf),
        "gru0": np.ascontiguousarray(gru0).astype(bf),
        "late": np.ascontiguousarray(late).astype(bf),
    }
    in_maps = []
    for ci in range(NCORES):
        nsh = np.asarray(nodes[ci * G:(ci + 1) * G], np.float32)   # [G,V,NF]
        esh = np.asarray(edges[ci * G:(ci + 1) * G], np.float32)   # [G,V,V,E]
        nodesT = np.ascontiguousarray(
            nsh.transpose(2, 0, 1).reshape(NF, VG)).astype(bf)
        # block-diagonal edges^T: [128(w), pair, E, 128(v)]; graph 2c+h's
        # edge matrix sits in rows/cols h*64:(h+1)*64, the rest is zero.
        et = esh.transpose(0, 2, 3, 1)                  # [G, w, e, v]
        edges_p = np.zeros((128, NPAIR, E, 128), np.float32)
        for c in range(NPAIR):
            edges_p[0:64, c, :, 0:64] = et[2 * c]
            edges_p[64:128, c, :, 64:128] = et[2 * c + 1]
        in_maps.append({"nodesT": nodesT,
                        "edges_p": edges_p.astype(f8), **shared})
    return in_maps


def kernel(**inputs):
    global LAST_RESULTS
    from concourse.bass_utils import run_bass_kernel_spmd

    nc = build()
    in_maps = make_in_maps(**inputs)
    res = run_bass_kernel_spmd(nc, in_maps, core_ids=list(range(NCORES)),
                               trace=TRACE)
    LAST_RESULTS = res
    # red[d, j, g] -> out[g, j*128 + d] per core
    return np.concatenate(
        [np.asarray(r["red"], np.float32).transpose(2, 1, 0).reshape(G, OUT)
         for r in res.results], axis=0)
